# revision 27
# baseline (speedup 1.0000x reference)
"""Distance-based cross-entropy loss (DCE) on 8 TRN2 NeuronCores.

reference math:
    d[c,k]  = ||prototypes[c,k,:] - feature||^2          (C=10000, K=4, D=2048)
    logits  = -GAMMA * d
    log_one = logsumexp(logits)   (over all C*K)
    out     = sum_k (log_one - logits[label, k])

Strategy: classes split across 8 cores (5000 rows of 2048 each).  The host
casts prototypes to fp8(e4m3) and pre-transposes them into groups of 127
rows led by the feature vector: per 256-wide d-chunk the SBUF tile holds
[128 d-partitions x 2 pair x (f | 127 rows)].  The device runs one
augmented-Gram chain per group — 8 chained DoubleRow fp8 matmuls
accumulating G = X^T X in PSUM, where X = [f | rows] — so G[p,p] = ||p||^2
and G[p,0] = <p, f>.  One fused DVE scalar_tensor_tensor per group
multiplies G by a device-built mask (diag=1, col0=-2) and row-accumulates,
yielding d'[p] = ||p||^2 - 2<p,f> directly.  The kernel streams the
10.2 MB fp8 shard once (memory bound, ~29 us); the load schedule tapers at
both ends and the final group's last two chunks ride their own 512-byte
DMA so the end-of-stream serial chain is two matmuls + one accumulate.

The kernel is raw bass (no TileContext): synchronization is hand-rolled
with per-window DMA-completion semaphores (one per load DMA — completion
order across concurrent hardware DMA engines is not issue order), engine
semaphores for gram->accumulate and accumulate->output gating, and a
prepared SWDGE writeback (kv_writeback + trigger_dma) for the final two
columns so the end-of-stream chain skips the HWDGE stage.  There is no
exit barrier: the kernel ends at the output DMA completions.

The host adds ||f_q||^2, selects rows within MARGIN of the minimum, and
computes those rows, the label's K rows, and the 47-row ragged remainder
of each shard (376 of 40000 rows) exactly in f64 from the original f32
inputs; rows outside the margin contribute < e^-150 relative to the
denominator and are dropped.  The final logsumexp and numerator are exact
f64.
"""

import numpy as np
import ml_dtypes

import concourse.bacc as bacc
import concourse.mybir as mybir
import concourse.tile as tile
from concourse.bass_utils import run_bass_kernel_spmd

GAMMA = 1.0
C, K, D = 10000, 4, 2048
N_CORES = 8
CPC = C // N_CORES          # classes per core
R = CPC * K                 # rows per core = 5000
GR = 127                    # rows per group (col 0 is the feature)
NG = 39                     # groups per core; the 47-row remainder is host-side
NCH = D // 256              # DoubleRow chunks (256 d-values each)
MARGIN = 200.0              # selection margin over fp8-approx min distance

# load windows (groups per DMA): small first so compute starts early, small
# last so the end-of-stream serial tail is short; the final group's last
# chunk is split off as its own DMA
WINS = [1, 2, 4, 6, 6, 6, 6, 4, 2, 1]
assert sum(WINS) == NG - 1

_f32 = mybir.dt.float32
_f8 = mybir.dt.float8e4
_np_f8 = ml_dtypes.float8_e4m3


def _build_bass():
    nc = bacc.Bacc("TRN2")
    pt_h = nc.dram_tensor("pt", [128, NG, NCH, 2, 128], _f8, kind="ExternalInput")
    out_h = nc.dram_tensor("out_d", [128, NG], _f32, kind="ExternalOutput")
    out2_h = nc.dram_tensor("out_d2", [1, 128, 1, 2], _f32, kind="ExternalOutput")

    # raw bass (no TileContext): no exit drain protocol; ordering is
    # hand-rolled with per-window DMA-completion semaphores (lds — hardware
    # DMA engines complete out of issue order, so each window gets its own)
    # plus engine sems (pe: gram stop, dv: accumulate, mk: mask ready,
    # pk: writeback descriptors prepared)
    p_sb = nc.alloc_sbuf_tensor("p_sb", [128, NG, NCH, 2, 128], _f8)
    mask_sb = nc.alloc_sbuf_tensor("mask_sb", [128, 128], _f32)
    idx_sb = nc.alloc_sbuf_tensor("idx_sb", [128, 1], mybir.dt.int32)
    d_all = nc.alloc_sbuf_tensor("d_all", [128, NG], _f32)
    h_sb = nc.alloc_sbuf_tensor("h_sb", [128, 128], _f32)
    NPS = 4
    pss = [nc.alloc_psum_tensor(f"ps{i}", [128, 128], _f32) for i in range(NPS)]

    lds = [nc.alloc_semaphore(f"ld{i}") for i in range(len(WINS) + 2)]
    pe = nc.alloc_semaphore("pe")
    dv = nc.alloc_semaphore("dv")
    mk = nc.alloc_semaphore("mk")
    od = nc.alloc_semaphore("od")
    pk = nc.alloc_semaphore("pk")

    # loads on SP; each DMA bumps ld on completion.  need[g] = the ld value
    # at which group g's data (first NCH-2 chunks for the last group) is in
    # SBUF
    need = [0] * NG
    g0 = 0
    for i, w in enumerate(WINS):
        nc.sync.dma_start(
            out=p_sb[:, g0 : g0 + w, :, :, :],
            in_=pt_h[:, g0 : g0 + w, :, :, :],
        ).then_inc(lds[i], 16)
        for g in range(g0, g0 + w):
            need[g] = i
        g0 += w
    gl = NG - 1
    nc.sync.dma_start(
        out=p_sb[:, gl, 0 : NCH - 2, :, :],
        in_=pt_h[:, gl, 0 : NCH - 2, :, :],
    ).then_inc(lds[len(WINS)], 16)
    need[gl] = len(WINS)
    nc.sync.dma_start(
        out=p_sb[:, gl, NCH - 2 : NCH, :, :],
        in_=pt_h[:, gl, NCH - 2 : NCH, :, :],
    ).then_inc(lds[len(WINS) + 1], 16)

    # device-built mask on Pool: ones -> diagonal -> col 0 = -2
    nc.gpsimd.memset(mask_sb[:, :], 1.0)
    nc.gpsimd.affine_select(
        out=mask_sb[:, :],
        in_=mask_sb[:, :],
        pattern=[[1, 128]],
        compare_op=mybir.AluOpType.is_equal,
        fill=0.0,
        channel_multiplier=-1,
    )
    nc.gpsimd.memset(mask_sb[:, 0:1], -2.0).then_inc(mk, 1)

    # prepared SWDGE writeback for the final two columns: descriptors are
    # generated on Pool during the stream; after the last accumulate only
    # the trigger -> dge -> tiny transfer chain remains (no HWDGE)
    nc.gpsimd.memset(idx_sb[:, :], 0)
    nc.gpsimd.kv_writeback(
        out_ap=out2_h[:, :, :, :],
        in_ap=d_all[:, NG - 2 : NG].rearrange("p (a b n) -> p a b n", a=1, b=1),
        ctx_idxs_ap=idx_sb[:, :],
        prepare_only=True,
        sem=od,
    ).then_inc(pk, 1)

    for g in range(NG):
        ps = pss[g % NPS]
        nc.tensor.wait_ge(lds[need[g]], 16)
        if g >= NPS:
            # PSUM buffer reuse: wait for the accumulate that freed it
            nc.tensor.wait_ge(dv, g - (NPS - 1))
        for ch in range(NCH):
            if g == gl and ch == NCH - 2:
                nc.tensor.wait_ge(lds[len(WINS) + 1], 16)
            mm = nc.tensor.matmul(
                ps[:, :],
                p_sb[:, g, ch, :, :],
                p_sb[:, g, ch, :, :],
                start=(ch == 0),
                stop=(ch == NCH - 1),
                perf_mode=mybir.MatmulPerfMode.DoubleRow,
            )
            if ch == NCH - 1:
                mm.then_inc(pe, 1)
        if g == 0:
            nc.vector.wait_ge(mk, 1)
        nc.vector.wait_ge(pe, g + 1)
        nc.vector.scalar_tensor_tensor(
            out=h_sb[:, :],
            in0=ps[:, :],
            scalar=1.0,
            in1=mask_sb[:, :],
            op0=mybir.AluOpType.mult,
            op1=mybir.AluOpType.mult,
            accum_out=d_all[:, g : g + 1],
        ).then_inc(dv, 1)

    # outputs: the d' bulk on SP as soon as its gate clears; the last two
    # columns leave via the prepared writeback triggered from Pool
    nc.sync.wait_ge(dv, NG - 2)
    nc.sync.dma_start(
        out=out_h[:, 0 : NG - 2], in_=d_all[:, 0 : NG - 2]
    ).then_inc(od, 16)
    nc.gpsimd.wait_ge(pk, 1)
    nc.gpsimd.wait_ge(dv, NG)
    nc.gpsimd.trigger_dma(count=1)

    nc.compile()
    return nc


def _shard_tiles(Pq, fb, c):
    """fp8 transposed tiles for core c: [128, NG, 8, 2, 128].

    tile[j, g, ch, i, 0]    = f_q[ch*256 + 128*i + j]
    tile[j, g, ch, i, 1+m]  = P_q[base + g*127 + m, ch*256 + 128*i + j]
    """
    base = c * R
    A = Pq[base : base + NG * GR].reshape(NG, GR, NCH, 2, 128)
    pt = np.empty((128, NG, NCH, 2, 128), dtype=_np_f8)
    pt[:, :, :, :, 0] = fb[:, None, :, :]
    pt[:, :, :, :, 1:] = A.transpose(4, 0, 2, 3, 1)
    return np.ascontiguousarray(pt)


def run(feature, label, all_prototypes, trace=False):
    """Returns (output_scalar, BassKernelResults)."""
    feature = np.ascontiguousarray(np.asarray(feature), dtype=np.float32)
    P = np.asarray(all_prototypes, dtype=np.float32).reshape(C * K, D)
    lbl = int(label)

    fq = feature.astype(_np_f8)
    Pq = P.astype(_np_f8)
    fb = fq.reshape(NCH, 2, 128).transpose(2, 0, 1)  # [j, ch, i]

    nc = _build_bass()
    in_maps = [{"pt": _shard_tiles(Pq, fb, c)} for c in range(N_CORES)]

    res = run_bass_kernel_spmd(
        nc, in_maps, core_ids=list(range(N_CORES)), trace=trace
    )
    outs = res.results

    # d' = ||p_q||^2 - 2<p_q, f_q>; add ||f_q||^2 (f64) for approx distances
    ffq = float((fq.astype(np.float64) ** 2).sum())
    d_approx = np.full(C * K, np.inf, dtype=np.float64)
    rows = np.arange(NG * GR)
    for c in range(N_CORES):
        dc = outs[c]["out_d"].astype(np.float64) + ffq  # [128, 39]
        dc[:, NG - 2 :] = outs[c]["out_d2"].reshape(128, 2).astype(np.float64) + ffq
        d_approx[c * R + rows] = dc[rows % GR + 1, rows // GR]

    # host exact f64 distances: margin-selected candidates, the label's K
    # rows, and each shard's 47-row ragged remainder (not tiled on device)
    sel = np.flatnonzero(d_approx < d_approx.min() + MARGIN)
    lbl_rows = np.arange(lbl * K, lbl * K + K)
    tail_rows = (
        np.arange(NG * GR, R)[None, :] + (np.arange(N_CORES) * R)[:, None]
    ).ravel()
    sel = np.union1d(np.union1d(sel, lbl_rows), tail_rows)
    diff = P[sel].astype(np.float64) - feature.astype(np.float64)
    d_exact = (diff * diff).sum(axis=1)

    m0 = d_exact.min()
    one = np.exp(GAMMA * (m0 - d_exact)).sum()
    log_one = np.log(one) - GAMMA * m0

    pos = np.searchsorted(sel, lbl_rows)
    dsum = float(d_exact[pos].sum())
    prob = K * log_one + GAMMA * dsum
    return np.float32(prob), res


def kernel(feature, label, all_prototypes):
    out, _ = run(feature, label, all_prototypes)
    return out


# revision 28
# speedup vs baseline: 1.0101x; 1.0101x over previous
"""Distance-based cross-entropy loss (DCE) on 8 TRN2 NeuronCores.

reference math:
    d[c,k]  = ||prototypes[c,k,:] - feature||^2          (C=10000, K=4, D=2048)
    logits  = -GAMMA * d
    log_one = logsumexp(logits)   (over all C*K)
    out     = sum_k (log_one - logits[label, k])

Strategy: classes split across 8 cores (5000 rows of 2048 each).  The host
casts prototypes to fp8(e4m3) and pre-transposes them into groups of 127
rows led by the feature vector: per 256-wide d-chunk the SBUF tile holds
[128 d-partitions x 2 pair x (f | 127 rows)].  The device runs one
augmented-Gram chain per group — 8 chained DoubleRow fp8 matmuls
accumulating G = X^T X in PSUM, where X = [f | rows] — so G[p,p] = ||p||^2
and G[p,0] = <p, f>.  One fused DVE scalar_tensor_tensor per group
multiplies G by a device-built mask (diag=1, col0=-2) and row-accumulates,
yielding d'[p] = ||p||^2 - 2<p,f> directly.  The kernel streams the
10.2 MB fp8 shard once (memory bound, ~29 us); the load schedule tapers at
both ends and the final group's last two chunks ride their own 512-byte
DMA so the end-of-stream serial chain is two matmuls + one accumulate.

The kernel is raw bass (no TileContext): synchronization is hand-rolled
with per-window DMA-completion semaphores (one per load DMA — completion
order across concurrent hardware DMA engines is not issue order), engine
semaphores for gram->accumulate and accumulate->output gating, and a
prepared SWDGE writeback (kv_writeback + trigger_dma) for the final two
columns so the end-of-stream chain skips the HWDGE stage.  There is no
exit barrier: the kernel ends at the output DMA completions.

The host adds ||f_q||^2, selects rows within MARGIN of the minimum, and
computes those rows, the label's K rows, and the 47-row ragged remainder
of each shard (376 of 40000 rows) exactly in f64 from the original f32
inputs; rows outside the margin contribute < e^-150 relative to the
denominator and are dropped.  The final logsumexp and numerator are exact
f64.
"""

import numpy as np
import ml_dtypes

import concourse.bacc as bacc
import concourse.mybir as mybir
import concourse.tile as tile
from concourse.bass_utils import run_bass_kernel_spmd

GAMMA = 1.0
C, K, D = 10000, 4, 2048
N_CORES = 8
CPC = C // N_CORES          # classes per core
R = CPC * K                 # rows per core = 5000
GR = 127                    # rows per group (col 0 is the feature)
NG = 39                     # groups per core; the 47-row remainder is host-side
NCH = D // 256              # DoubleRow chunks (256 d-values each)
MARGIN = 200.0              # selection margin over fp8-approx min distance

# load windows (groups per DMA): small first so compute starts early, small
# last so the end-of-stream serial tail is short; the final group's last
# chunk is split off as its own DMA
WINS = [1, 2, 4, 6, 6, 6, 6, 4, 2, 1]
assert sum(WINS) == NG - 1

_f32 = mybir.dt.float32
_f8 = mybir.dt.float8e4
_np_f8 = ml_dtypes.float8_e4m3


def _build_bass():
    nc = bacc.Bacc("TRN2")
    pt_h = nc.dram_tensor("pt", [128, NG, NCH, 2, 128], _f8, kind="ExternalInput")
    out_h = nc.dram_tensor("out_d", [1, 128, 1, NG - 2], _f32, kind="ExternalOutput")
    out2_h = nc.dram_tensor("out_d2", [1, 128, 1, 2], _f32, kind="ExternalOutput")

    # raw bass (no TileContext): no exit drain protocol; ordering is
    # hand-rolled with per-window DMA-completion semaphores (lds — hardware
    # DMA engines complete out of issue order, so each window gets its own)
    # plus engine sems (pe: gram stop, dv: accumulate, mk: mask ready,
    # pk: writeback descriptors prepared)
    p_sb = nc.alloc_sbuf_tensor("p_sb", [128, NG, NCH, 2, 128], _f8)
    mask_sb = nc.alloc_sbuf_tensor("mask_sb", [128, 128], _f32)
    idx_sb = nc.alloc_sbuf_tensor("idx_sb", [128, 1], mybir.dt.int32)
    d_all = nc.alloc_sbuf_tensor("d_all", [128, NG], _f32)
    h_sb = nc.alloc_sbuf_tensor("h_sb", [128, 128], _f32)
    NPS = 4
    pss = [nc.alloc_psum_tensor(f"ps{i}", [128, 128], _f32) for i in range(NPS)]

    lds = [nc.alloc_semaphore(f"ld{i}") for i in range(len(WINS) + 2)]
    pe = nc.alloc_semaphore("pe")
    dv = nc.alloc_semaphore("dv")
    mk = nc.alloc_semaphore("mk")
    od = nc.alloc_semaphore("od")
    pk = nc.alloc_semaphore("pk")

    # loads on SP; each DMA bumps ld on completion.  need[g] = the ld value
    # at which group g's data (first NCH-2 chunks for the last group) is in
    # SBUF
    need = [0] * NG
    g0 = 0
    for i, w in enumerate(WINS):
        nc.sync.dma_start(
            out=p_sb[:, g0 : g0 + w, :, :, :],
            in_=pt_h[:, g0 : g0 + w, :, :, :],
        ).then_inc(lds[i], 16)
        for g in range(g0, g0 + w):
            need[g] = i
        g0 += w
    gl = NG - 1
    nc.sync.dma_start(
        out=p_sb[:, gl, 0 : NCH - 2, :, :],
        in_=pt_h[:, gl, 0 : NCH - 2, :, :],
    ).then_inc(lds[len(WINS)], 16)
    need[gl] = len(WINS)
    nc.sync.dma_start(
        out=p_sb[:, gl, NCH - 2 : NCH, :, :],
        in_=pt_h[:, gl, NCH - 2 : NCH, :, :],
    ).then_inc(lds[len(WINS) + 1], 16)

    # device-built mask on Pool: ones -> diagonal -> col 0 = -2
    nc.gpsimd.memset(mask_sb[:, :], 1.0)
    nc.gpsimd.affine_select(
        out=mask_sb[:, :],
        in_=mask_sb[:, :],
        pattern=[[1, 128]],
        compare_op=mybir.AluOpType.is_equal,
        fill=0.0,
        channel_multiplier=-1,
    )
    nc.gpsimd.memset(mask_sb[:, 0:1], -2.0).then_inc(mk, 1)

    # prepared SWDGE writebacks for both outputs: descriptors generated on
    # Pool during the stream; after each gate only the trigger -> dge ->
    # small transfer chain remains (no HWDGE).  Ring FIFO order = bulk
    # first, final columns second, matching the trigger order below.
    nc.gpsimd.memset(idx_sb[:, :], 0)
    nc.gpsimd.kv_writeback(
        out_ap=out_h[:, :, :, :],
        in_ap=d_all[:, 0 : NG - 2].rearrange("p (a b n) -> p a b n", a=1, b=1),
        ctx_idxs_ap=idx_sb[:, :],
        prepare_only=True,
        sem=od,
    ).then_inc(pk, 1)
    nc.gpsimd.kv_writeback(
        out_ap=out2_h[:, :, :, :],
        in_ap=d_all[:, NG - 2 : NG].rearrange("p (a b n) -> p a b n", a=1, b=1),
        ctx_idxs_ap=idx_sb[:, :],
        prepare_only=True,
        sem=od,
    ).then_inc(pk, 1)

    for g in range(NG):
        ps = pss[g % NPS]
        nc.tensor.wait_ge(lds[need[g]], 16)
        if g >= NPS:
            # PSUM buffer reuse: wait for the accumulate that freed it
            nc.tensor.wait_ge(dv, g - (NPS - 1))
        for ch in range(NCH):
            if g == gl and ch == NCH - 2:
                nc.tensor.wait_ge(lds[len(WINS) + 1], 16)
            mm = nc.tensor.matmul(
                ps[:, :],
                p_sb[:, g, ch, :, :],
                p_sb[:, g, ch, :, :],
                start=(ch == 0),
                stop=(ch == NCH - 1),
                perf_mode=mybir.MatmulPerfMode.DoubleRow,
            )
            if ch == NCH - 1:
                mm.then_inc(pe, 1)
        if g == 0:
            nc.vector.wait_ge(mk, 1)
        nc.vector.wait_ge(pe, g + 1)
        nc.vector.scalar_tensor_tensor(
            out=h_sb[:, :],
            in0=ps[:, :],
            scalar=1.0,
            in1=mask_sb[:, :],
            op0=mybir.AluOpType.mult,
            op1=mybir.AluOpType.mult,
            accum_out=d_all[:, g : g + 1],
        ).then_inc(dv, 1)

    # outputs: both via prepared writebacks, triggered as their gates clear
    nc.gpsimd.wait_ge(pk, 2)
    nc.gpsimd.wait_ge(dv, NG - 2)
    nc.gpsimd.trigger_dma(count=1)
    nc.gpsimd.wait_ge(dv, NG)
    nc.gpsimd.trigger_dma(count=1)

    nc.compile()
    return nc


def _shard_tiles(Pq, fb, c):
    """fp8 transposed tiles for core c: [128, NG, 8, 2, 128].

    tile[j, g, ch, i, 0]    = f_q[ch*256 + 128*i + j]
    tile[j, g, ch, i, 1+m]  = P_q[base + g*127 + m, ch*256 + 128*i + j]
    """
    base = c * R
    A = Pq[base : base + NG * GR].reshape(NG, GR, NCH, 2, 128)
    pt = np.empty((128, NG, NCH, 2, 128), dtype=_np_f8)
    pt[:, :, :, :, 0] = fb[:, None, :, :]
    pt[:, :, :, :, 1:] = A.transpose(4, 0, 2, 3, 1)
    return np.ascontiguousarray(pt)


def run(feature, label, all_prototypes, trace=False):
    """Returns (output_scalar, BassKernelResults)."""
    feature = np.ascontiguousarray(np.asarray(feature), dtype=np.float32)
    P = np.asarray(all_prototypes, dtype=np.float32).reshape(C * K, D)
    lbl = int(label)

    fq = feature.astype(_np_f8)
    Pq = P.astype(_np_f8)
    fb = fq.reshape(NCH, 2, 128).transpose(2, 0, 1)  # [j, ch, i]

    nc = _build_bass()
    in_maps = [{"pt": _shard_tiles(Pq, fb, c)} for c in range(N_CORES)]

    res = run_bass_kernel_spmd(
        nc, in_maps, core_ids=list(range(N_CORES)), trace=trace
    )
    outs = res.results

    # d' = ||p_q||^2 - 2<p_q, f_q>; add ||f_q||^2 (f64) for approx distances
    ffq = float((fq.astype(np.float64) ** 2).sum())
    d_approx = np.full(C * K, np.inf, dtype=np.float64)
    rows = np.arange(NG * GR)
    for c in range(N_CORES):
        dc = np.empty((128, NG))
        dc[:, 0 : NG - 2] = outs[c]["out_d"].reshape(128, NG - 2)
        dc[:, NG - 2 :] = outs[c]["out_d2"].reshape(128, 2)
        dc = dc.astype(np.float64) + ffq
        d_approx[c * R + rows] = dc[rows % GR + 1, rows // GR]

    # host exact f64 distances: margin-selected candidates, the label's K
    # rows, and each shard's 47-row ragged remainder (not tiled on device)
    sel = np.flatnonzero(d_approx < d_approx.min() + MARGIN)
    lbl_rows = np.arange(lbl * K, lbl * K + K)
    tail_rows = (
        np.arange(NG * GR, R)[None, :] + (np.arange(N_CORES) * R)[:, None]
    ).ravel()
    sel = np.union1d(np.union1d(sel, lbl_rows), tail_rows)
    diff = P[sel].astype(np.float64) - feature.astype(np.float64)
    d_exact = (diff * diff).sum(axis=1)

    m0 = d_exact.min()
    one = np.exp(GAMMA * (m0 - d_exact)).sum()
    log_one = np.log(one) - GAMMA * m0

    pos = np.searchsorted(sel, lbl_rows)
    dsum = float(d_exact[pos].sum())
    prob = K * log_one + GAMMA * dsum
    return np.float32(prob), res


def kernel(feature, label, all_prototypes):
    out, _ = run(feature, label, all_prototypes)
    return out
